# revision 16
# baseline (speedup 1.0000x reference)
"""GNN message-passing kernel for Trainium2 (8 NeuronCores, SPMD).

Strategy (edge-parallel by destination):
  * Host sorts edges by destination 128-node block, assigns blocks to
    (core, slot) pairs so per-slot edge counts balance across cores (one
    shared compile-time tile schedule for all 8 cores).
  * Host pre-gathers [x[row] | edge_attr | 1] per edge, scales each
    edge column by wrc = wts/max(cnt,1) (relu commutes with the positive
    scale, so the weighted-mean normalization folds into the fp8 eax
    stream for free), and precomputes the one-hot scatter tiles S as an
    fp8 {0,1} stream.
  * Bulk streams (eax, S) ride the gpsimd SWDGE queue, whose descriptors
    spread across all 16 SDMA engines; the small node streams
    (xua/xub/out) ride the sync/scalar HWDGE queues (single-engine each).
  * Device, per 128-edge tile: h = eaxT.T @ mw1 (PSUM, 4 tiles share a
    [128,512] bank), G = relu(h) (one op per 4 tiles, cycled over
    ACT/DVE), T_w[hid, node] += G.T @ S accumulated in PSUM across a
    4-window [128, 512] bank.  No per-tile DVE work remains.
  * Per 4-window group: recvT = mw2.T @ T4 (N=512), update MLP with
    biases folded into augmented weight rows, out written transposed in
    bf16.
  * Host inverts the node permutation and returns [N, 64] float32.
"""
import numpy as np
import ml_dtypes

import concourse.bacc as bacc
import concourse.tile as tile
from concourse import mybir
from concourse.bass_utils import run_bass_kernel_spmd

BF = mybir.dt.bfloat16
F8 = mybir.dt.float8e4
F32 = mybir.dt.float32
bf16 = ml_dtypes.bfloat16
f8 = ml_dtypes.float8_e4m3fn

P = 128
NCORES = 8
HID = 128
NODE_D = 64
EDGE_D = 32
GLOB_D = 32
FEAT = NODE_D + EDGE_D + 1   # 97: x | edge_attr | ones
UPB = GLOB_D + 2             # 34: u | s | ones

# const blob column layout (bf16, [128, 768])
_B_MW1 = 0      # [0:97, 0:128]    mw1_aug
_B_MW2 = 128    # [0:128, 128:192] mw2
_B_UW1A = 192   # [0:128, 192:320] uw1 rows 0:128 (x part | recv part)
_B_UW1R = 320   # [0:64, 320:448]  uw1 rows 64:128 (recv part, base partition 0)
_B_UW1B = 448   # [0:34, 448:576]  uw1 rows 128:160 (u part) | v | ub1
_B_UW2 = 576    # [0:128, 576:640] uw2
_B_IOTA = 640   # [0:128, 640:768] iota row 0..127
BLOB_W = 768

CFG = {
    "group": 64,      # 128-edge tiles per eax/S DMA
    "agrp": 4,        # tiles sharing one PSUM h bank + one relu
    "wn": 128,        # nodes per scatter window
    "wgrp": 4,        # windows per scatter bank / update-MLP batch (wn*wgrp=512)
    "eax_bufs": 6,
    "s_bufs": 6,
    "gx_bufs": 3,
    "gout_bufs": 3,
    "ework": 4,
    "swork": 8,
    "nwork": 2,
    "ph": 3, "pt": 3,
    "s_mode": "stream",            # stream | dve
    "eax_q": ("gpsimd",),          # queue cycle for eax groups
    "s_q": ("gpsimd",),            # queue cycle for S groups
    "xua_q": "scalar", "xub_q": "scalar", "out_q": "sync",
    "relu_cycle": ("scalar", "scalar", "vector"),
    "hh_eng": "vector", "recv_eng": "vector", "h2r_eng": "vector",
    "obias_eng": "scalar",
}

_program_cache: dict = {}
_last_results = None


def _copy(nc, eng, out, in_):
    if eng == "vector":
        nc.vector.tensor_copy(out, in_)
    else:
        nc.scalar.copy(out, in_)


def _relu(nc, eng, out, in_):
    if eng == "vector":
        nc.vector.tensor_scalar_max(out, in_, 0.0)
    else:
        nc.scalar.activation(out, in_, mybir.ActivationFunctionType.Relu)


def _build_program(t_sched):
    nt = sum(t_sched)
    e_pad = nt * P
    nslots = len(t_sched)
    nsh = nslots * CFG["wn"]
    GROUP = CFG["group"]
    AGRP = CFG["agrp"]
    WN = CFG["wn"]
    WGRP = CFG["wgrp"]
    assert WN * WGRP == 512
    stream_s = CFG["s_mode"] == "stream"
    assert nt % 32 == 0
    gsizes = [8, 8, 8, 8]
    rem = nt - 32
    while rem > 0:
        gsz = min(GROUP, rem)
        gsizes.append(gsz)
        rem -= gsz
    assert sum(gsizes) == nt
    g_of = []
    gstart = []
    off = 0
    for gi, gsz in enumerate(gsizes):
        gstart.append(off)
        for r in range(gsz):
            g_of.append((gi, r))
        off += gsz

    nc = bacc.Bacc()
    eax_d = nc.dram_tensor("eax", [FEAT, e_pad], F8, kind="ExternalInput")
    if stream_s:
        s_d = nc.dram_tensor("s", [P, nt * CFG["wn"]], F8, kind="ExternalInput")
    else:
        cw_d = nc.dram_tensor("cw", [P, nt], F32, kind="ExternalInput")
    blob_d = nc.dram_tensor("blob", [P, BLOB_W], BF, kind="ExternalInput")
    ub2_d = nc.dram_tensor("ub2", [64, 1], F32, kind="ExternalInput")
    xua_d = nc.dram_tensor("xua", [NODE_D, nsh], BF, kind="ExternalInput")
    xub_d = nc.dram_tensor("xub", [UPB, nsh], BF, kind="ExternalInput")
    out_d = nc.dram_tensor("out", [64, nsh], BF, kind="ExternalOutput")

    with tile.TileContext(nc) as tc:
        with (
            tc.tile_pool(name="consts", bufs=1) as consts,
            tc.tile_pool(name="geax", bufs=CFG["eax_bufs"]) as geax,
            tc.tile_pool(name="gs", bufs=CFG["s_bufs"]) as gs,
            tc.tile_pool(name="gx", bufs=CFG["gx_bufs"]) as gx,
            tc.tile_pool(name="ework", bufs=CFG["ework"]) as ework,
            tc.tile_pool(name="swork", bufs=CFG["swork"]) as swork,
            tc.tile_pool(name="nwork", bufs=CFG["nwork"]) as nwork,
            tc.tile_pool(name="gout", bufs=CFG["gout_bufs"]) as gout,
            tc.tile_pool(name="ph", bufs=CFG["ph"], space="PSUM") as ph,
            tc.tile_pool(name="pt", bufs=CFG["pt"], space="PSUM") as pt,
            tc.tile_pool(name="pr", bufs=1, space="PSUM") as pr,
            tc.tile_pool(name="p2", bufs=1, space="PSUM") as p2,
        ):
            blob_t = consts.tile([P, BLOB_W], BF)
            nc.sync.dma_start(blob_t[:], blob_d[:])
            mw1_t = blob_t[0:FEAT, _B_MW1:_B_MW1 + HID]
            mw2_t = blob_t[0:HID, _B_MW2:_B_MW2 + 64]
            uw1ax_t = blob_t[0:NODE_D, _B_UW1A:_B_UW1A + HID]
            uw1ar_t = blob_t[0:NODE_D, _B_UW1R:_B_UW1R + HID]
            uw1b_t = blob_t[0:UPB, _B_UW1B:_B_UW1B + HID]
            uw2_t = blob_t[0:HID, _B_UW2:_B_UW2 + 64]
            iota_t = blob_t[:, _B_IOTA:_B_IOTA + P]
            ub2_t = consts.tile([64, 1], F32)
            nc.sync.dma_start(ub2_t[:], ub2_d[:])
            if not stream_s:
                cw_t = consts.tile([P, nt], F32)
                nc.scalar.dma_start(cw_t[:], cw_d[:])

            eaxq = CFG["eax_q"]
            sq = CFG["s_q"]
            rcycle = CFG["relu_cycle"]
            eax_g = s_g = None
            xua_g = xub_g = o_g = None
            t4_ps = None
            t = 0
            rr = 0
            for j in range(nslots):
                tj = t_sched[j]
                jg, jr = divmod(j, WGRP)
                if jr == 0:
                    nw = min(WGRP, nslots - jg * WGRP)
                    W = nw * WN
                    xua_g = gx.tile([NODE_D, WGRP * WN], BF, tag="xua")
                    getattr(nc, CFG["xua_q"]).dma_start(
                        xua_g[:, 0:W],
                        xua_d[:, jg * WGRP * WN:jg * WGRP * WN + W],
                    )

                    xub_g = gx.tile([UPB, WGRP * WN], BF, tag="xub")
                    getattr(nc, CFG["xub_q"]).dma_start(
                        xub_g[:, 0:W],
                        xub_d[:, jg * WGRP * WN:jg * WGRP * WN + W],
                    )
                    o_g = gout.tile([64, WGRP * WN], BF, tag="o")
                    t4_ps = pt.tile([HID, WGRP * WN], F32, space="PSUM")
                kdone = 0
                while kdone < tj:
                    nk = min(AGRP, tj - kdone)
                    h4_ps = ph.tile([P, AGRP * HID], F32, space="PSUM")
                    chunk = []
                    for i in range(nk):
                        g, r = g_of[t]
                        if r == 0:
                            n = gsizes[g]
                            g0 = gstart[g]
                            eax_g = geax.tile([FEAT, GROUP * P], F8, tag="eax")
                            eng = getattr(nc, eaxq[g % len(eaxq)])
                            eng.dma_start(
                                eax_g[:, 0:n * P],
                                eax_d[:, g0 * P:(g0 + n) * P],
                            )
                            if stream_s:
                                s_g = gs.tile([P, GROUP * WN], F8, tag="sg")
                                eng = getattr(nc, sq[g % len(sq)])
                                eng.dma_start(
                                    s_g[:, 0:n * WN],
                                    s_d[:, g0 * WN:(g0 + n) * WN],
                                )
                        nc.tensor.matmul(
                            h4_ps[:, i * HID:(i + 1) * HID],
                            lhsT=eax_g[:, r * P:(r + 1) * P], rhs=mw1_t,
                            start=True, stop=True,
                        )
                        if stream_s:
                            s_t = s_g[:, r * WN:(r + 1) * WN]
                        else:
                            s_t = swork.tile([P, WN], BF, tag="S")
                            nc.vector.tensor_scalar(
                                out=s_t[:], in0=iota_t[:, 0:WN],
                                scalar1=cw_t[:, t:t + 1], scalar2=0.0,
                                op0=mybir.AluOpType.is_equal,
                                op1=mybir.AluOpType.bypass,
                            )
                            s_t = s_t[:]
                        chunk.append((i, s_t))
                        t += 1
                    g4_t = ework.tile([P, AGRP * HID], F8, tag="G")
                    _relu(nc, rcycle[rr % len(rcycle)],
                          g4_t[:, 0:nk * HID], h4_ps[:, 0:nk * HID])
                    rr += 1
                    for i, s_t in chunk:
                        nc.tensor.matmul(
                            t4_ps[:, jr * WN:(jr + 1) * WN],
                            lhsT=g4_t[:, i * HID:(i + 1) * HID],
                            rhs=s_t,
                            start=(kdone + i == 0), stop=(kdone + i == tj - 1),
                        )
                    kdone += nk
                if jr == nw - 1:
                    W = nw * WN
                    hh = nwork.tile([HID, WGRP * WN], BF, tag="Hh")
                    _copy(nc, CFG["hh_eng"], hh[:, 0:W], t4_ps[:, 0:W])
                    r4_ps = pr.tile([64, WGRP * WN], F32, space="PSUM", tag="r4")
                    nc.tensor.matmul(r4_ps[:, 0:W], lhsT=mw2_t, rhs=hh[:, 0:W],
                                     start=True, stop=True)
                    recv_sb = nwork.tile([64, WGRP * WN], BF, tag="recv")
                    _copy(nc, CFG["recv_eng"], recv_sb[:, 0:W], r4_ps[:, 0:W])
                    h2_ps = p2.tile([HID, WGRP * WN], F32, space="PSUM")
                    nc.tensor.matmul(h2_ps[:, 0:W], lhsT=uw1ax_t,
                                     rhs=xua_g[:, 0:W], start=True, stop=False)
                    nc.tensor.matmul(h2_ps[:, 0:W], lhsT=uw1ar_t,
                                     rhs=recv_sb[:, 0:W], start=False,
                                     stop=False)
                    nc.tensor.matmul(h2_ps[:, 0:W], lhsT=uw1b_t,
                                     rhs=xub_g[:, 0:W], start=False, stop=True)
                    h2r = nwork.tile([HID, WGRP * WN], BF, tag="h2r")
                    _relu(nc, CFG["h2r_eng"], h2r[:, 0:W], h2_ps[:, 0:W])
                    o_ps = pr.tile([64, WGRP * WN], F32, space="PSUM", tag="r4")
                    nc.tensor.matmul(o_ps[:, 0:W], lhsT=uw2_t, rhs=h2r[:, 0:W],
                                     start=True, stop=True)
                    if CFG["obias_eng"] == "scalar":
                        nc.scalar.activation(
                            o_g[:, 0:W], o_ps[:, 0:W],
                            mybir.ActivationFunctionType.Identity,
                            bias=ub2_t[:, 0:1],
                        )
                    else:
                        nc.vector.tensor_scalar(
                            out=o_g[:, 0:W], in0=o_ps[:, 0:W],
                            scalar1=ub2_t[:, 0:1], scalar2=None,
                            op0=mybir.AluOpType.add,
                        )
                    getattr(nc, CFG["out_q"]).dma_start(
                        out_d[:, jg * WGRP * WN:jg * WGRP * WN + W],
                        o_g[:, 0:W],
                    )
    nc.finalize()
    return nc


def _schedule(col, n_nodes):
    """Assign WN-node blocks to (core, slot); shared per-slot tile counts."""
    GROUP = CFG["group"]
    WN = CFG["wn"]
    WSH = WN.bit_length() - 1
    nblk = -(-n_nodes // WN)
    nslots = -(-nblk // NCORES)
    nblk_pad = nslots * NCORES
    nsh = nslots * WN

    blk = (col >> WSH).astype(np.int64)
    order = np.argsort(blk, kind="stable")
    bc = np.bincount(blk, minlength=nblk_pad)
    bstart = np.zeros(nblk_pad + 1, np.int64)
    np.cumsum(bc, out=bstart[1:])

    sorted_blocks = np.argsort(-bc, kind="stable")
    blk_assign = sorted_blocks.reshape(nslots, NCORES)   # [slot, core]
    grp_max = bc[blk_assign].max(axis=1)
    t_sched = np.maximum(1, -(-grp_max // P)).astype(np.int64)
    pad = (-int(t_sched.sum())) % 32
    t_sched[-1] += pad
    t_sched = [int(v) for v in t_sched]
    return t_sched, blk_assign, order, bc, bstart, nslots, nsh


def kernel(x, edge_index, edge_attr, u, node_batch, wts,
           mw1, mb1, mw2, mb2, uw1, ub1, uw2, ub2):
    x = np.asarray(x, np.float32)
    edge_index = np.asarray(edge_index)
    edge_attr = np.asarray(edge_attr, np.float32)
    u = np.asarray(u, np.float32)
    node_batch = np.asarray(node_batch).astype(np.int64)
    wts = np.asarray(wts, np.float32).reshape(-1)
    mw1 = np.asarray(mw1, np.float32)
    mb1 = np.asarray(mb1, np.float32)
    mw2 = np.asarray(mw2, np.float32)
    mb2 = np.asarray(mb2, np.float32)
    uw1 = np.asarray(uw1, np.float32)
    ub1 = np.asarray(ub1, np.float32)
    uw2 = np.asarray(uw2, np.float32)
    ub2 = np.asarray(ub2, np.float32)

    n_nodes = x.shape[0]
    row = np.asarray(edge_index[0], np.int64)
    col = np.asarray(edge_index[1], np.int64)

    sched = _schedule(col, n_nodes)
    (t_sched, blk_assign, order, bc, bstart, nslots, nsh) = sched
    nt = sum(t_sched)
    e_pad = nt * P
    stream_s = CFG["s_mode"] == "stream"

    cnt = np.bincount(col, minlength=n_nodes).astype(np.float32)
    rc = 1.0 / np.maximum(cnt, 1.0)
    wsum = np.bincount(col, weights=wts, minlength=n_nodes).astype(np.float32)
    s_node = wsum * rc

    WN = CFG["wn"]
    colof = (col & (WN - 1)).astype(np.int64)
    wrc = wts * rc[col]

    key = (tuple(t_sched), CFG["s_mode"])
    if key not in _program_cache:
        _program_cache[key] = _build_program(t_sched)
    nc = _program_cache[key]

    # const blob (shared by all cores)
    v_row = mb2 @ uw1[NODE_D:2 * NODE_D, :]              # [HID]
    blob = np.zeros((P, BLOB_W), np.float32)
    blob[0:NODE_D + EDGE_D, _B_MW1:_B_MW1 + HID] = mw1
    blob[NODE_D + EDGE_D, _B_MW1:_B_MW1 + HID] = mb1
    blob[0:HID, _B_MW2:_B_MW2 + 64] = mw2
    blob[0:2 * NODE_D, _B_UW1A:_B_UW1A + HID] = uw1[0:2 * NODE_D, :]
    blob[0:NODE_D, _B_UW1R:_B_UW1R + HID] = uw1[NODE_D:2 * NODE_D, :]
    blob[0:GLOB_D, _B_UW1B:_B_UW1B + HID] = uw1[2 * NODE_D:, :]
    blob[GLOB_D, _B_UW1B:_B_UW1B + HID] = v_row
    blob[GLOB_D + 1, _B_UW1B:_B_UW1B + HID] = ub1
    blob[0:HID, _B_UW2:_B_UW2 + 64] = uw2
    blob[:, _B_IOTA:_B_IOTA + P] = np.arange(P, dtype=np.float32)[None, :]
    blob_bf = blob.astype(bf16)
    ub2_a = ub2.reshape(64, 1).astype(np.float32)

    u_per_node = u[node_batch]                           # [N, GLOB_D]

    slot_off = np.zeros(nslots + 1, np.int64)
    np.cumsum(np.asarray(t_sched) * P, out=slot_off[1:])

    tidx_all = np.arange(e_pad) // P
    pidx_all = np.arange(e_pad) % P

    in_maps = []
    node_idx_cores = []
    for c in range(NCORES):
        eidx = np.full(e_pad, -1, np.int64)
        nidx = np.full(nsh, -1, np.int64)
        for j in range(nslots):
            b = int(blk_assign[j, c])
            m = int(bc[b])
            o = slot_off[j]
            eidx[o:o + m] = order[bstart[b]:bstart[b] + m]
            n0 = b * WN
            nn = min(WN, n_nodes - n0)
            if nn > 0:
                nidx[j * WN:j * WN + nn] = np.arange(n0, n0 + nn)
        evalid = eidx >= 0
        eidxc = np.where(evalid, eidx, 0)
        # eax: [x[row] | edge_attr | 1] * wrc, transposed, fp8
        eax = np.empty((e_pad, FEAT), np.float32)
        eax[:, 0:NODE_D] = x[row[eidxc]]
        eax[:, NODE_D:NODE_D + EDGE_D] = edge_attr[eidxc]
        eax[:, FEAT - 1] = 1.0
        wrcv = np.zeros(e_pad, np.float32)
        wrcv[evalid] = wrc[eidx[evalid]]
        eax *= wrcv[:, None]
        np.clip(eax, -240.0, 240.0, out=eax)

        im = {
            "eax": np.ascontiguousarray(eax.T).astype(f8),
            "blob": blob_bf,
            "ub2": ub2_a,
        }
        if stream_s:
            s_all = np.zeros((nt, P, WN), f8)
            cv = colof[eidx[evalid]]
            s_all[tidx_all[evalid], pidx_all[evalid], cv] = f8(1.0)
            im["s"] = np.ascontiguousarray(
                s_all.transpose(1, 0, 2).reshape(P, nt * WN))
        else:
            cwv = np.zeros(e_pad, np.float32)
            cwv[evalid] = colof[eidx[evalid]].astype(np.float32)
            cwv[~evalid] = -1.0
            im["cw"] = np.ascontiguousarray(
                cwv.reshape(nt, P).T).astype(np.float32)

        nvalid = nidx >= 0
        nidxc = np.where(nvalid, nidx, 0)
        xua = x[nidxc].astype(np.float32)
        xua[~nvalid] = 0.0
        xub = np.zeros((nsh, UPB), np.float32)
        xub[:, 0:GLOB_D] = u_per_node[nidxc]
        xub[:, GLOB_D] = s_node[nidxc]
        xub[:, GLOB_D + 1] = 1.0
        xub[~nvalid] = 0.0
        im["xua"] = np.ascontiguousarray(xua.T).astype(bf16)
        im["xub"] = np.ascontiguousarray(xub.T).astype(bf16)

        in_maps.append(im)
        node_idx_cores.append((nidx, nvalid))

    res = run_bass_kernel_spmd(nc, in_maps, core_ids=list(range(NCORES)))
    global _last_results
    _last_results = res

    out_full = np.zeros((n_nodes, 64), np.float32)
    for c in range(NCORES):
        nidx, nvalid = node_idx_cores[c]
        oc = np.asarray(res.results[c]["out"], dtype=np.float32)  # [64, nsh]
        out_full[nidx[nvalid]] = oc.T[nvalid]
    return out_full


# revision 17
# speedup vs baseline: 1.0500x; 1.0500x over previous
"""GNN message-passing kernel for Trainium2 (8 NeuronCores, SPMD).

Strategy (edge-parallel by destination):
  * Host sorts edges by destination 128-node block, assigns blocks to
    (core, slot) pairs so per-slot edge counts balance across cores (one
    shared compile-time tile schedule for all 8 cores).
  * Host pre-gathers [x[row] | edge_attr | 1] per edge, scales each
    edge column by wrc = wts/max(cnt,1) (relu commutes with the positive
    scale, so the weighted-mean normalization folds into the fp8 eax
    stream for free), and precomputes the one-hot scatter tiles S as an
    fp8 {0,1} stream.
  * Bulk streams (eax, S) ride the gpsimd SWDGE queue, whose descriptors
    spread across all 16 SDMA engines; the small node streams
    (xua/xub/out) ride the sync/scalar HWDGE queues (single-engine each).
  * Device, per 128-edge tile: h = eaxT.T @ mw1 (PSUM, 4 tiles share a
    [128,512] bank), G = relu(h) (one op per 4 tiles, cycled over
    ACT/DVE), T_w[hid, node] += G.T @ S accumulated in PSUM across a
    4-window [128, 512] bank.  No per-tile DVE work remains.
  * Per 4-window group: recvT = mw2.T @ T4 (N=512), update MLP with
    biases folded into augmented weight rows, out written transposed in
    bf16.
  * Host inverts the node permutation and returns [N, 64] float32.
"""
import numpy as np
import ml_dtypes

import concourse.bacc as bacc
import concourse.tile as tile
from concourse import mybir
from concourse.bass_utils import run_bass_kernel_spmd

BF = mybir.dt.bfloat16
F8 = mybir.dt.float8e4
F32 = mybir.dt.float32
bf16 = ml_dtypes.bfloat16
f8 = ml_dtypes.float8_e4m3fn

P = 128
NCORES = 8
HID = 128
NODE_D = 64
EDGE_D = 32
GLOB_D = 32
FEAT = NODE_D + EDGE_D + 1   # 97: x | edge_attr | ones
UPB = GLOB_D + 2             # 34: u | s | ones

# const blob column layout (bf16, [128, 768])
_B_MW1 = 0      # [0:97, 0:128]    mw1_aug
_B_MW2 = 128    # [0:128, 128:192] mw2
_B_UW1A = 192   # [0:128, 192:320] uw1 rows 0:128 (x part | recv part)
_B_UW1R = 320   # [0:64, 320:448]  uw1 rows 64:128 (recv part, base partition 0)
_B_UW1B = 448   # [0:34, 448:576]  uw1 rows 128:160 (u part) | v | ub1
_B_UW2 = 576    # [0:128, 576:640] uw2
_B_IOTA = 640   # [0:128, 640:768] iota row 0..127
BLOB_W = 768

CFG = {
    "group": 32,      # 128-edge tiles per eax/S DMA
    "agrp": 4,        # tiles sharing one PSUM h bank + one relu
    "wn": 128,        # nodes per scatter window
    "wgrp": 4,        # windows per scatter bank / update-MLP batch (wn*wgrp=512)
    "eax_bufs": 6,
    "s_bufs": 6,
    "gx_bufs": 3,
    "gout_bufs": 3,
    "ework": 4,
    "swork": 8,
    "nwork": 2,
    "ph": 3, "pt": 2,
    "s_mode": "stream",            # stream | dve
    "eax_q": ("gpsimd",),          # queue cycle for eax groups
    "s_q": ("gpsimd",),            # queue cycle for S groups
    "xua_q": "scalar", "xub_q": "scalar", "out_q": "sync",
    "relu_cycle": ("scalar", "scalar", "vector"),
    "hh_eng": "vector", "recv_eng": "vector", "h2r_eng": "vector",
    "obias_eng": "scalar",
}

_program_cache: dict = {}
_last_results = None


def _copy(nc, eng, out, in_):
    if eng == "vector":
        nc.vector.tensor_copy(out, in_)
    else:
        nc.scalar.copy(out, in_)


def _relu(nc, eng, out, in_):
    if eng == "vector":
        nc.vector.tensor_scalar_max(out, in_, 0.0)
    else:
        nc.scalar.activation(out, in_, mybir.ActivationFunctionType.Relu)


def _build_program(t_sched):
    nt = sum(t_sched)
    e_pad = nt * P
    nslots = len(t_sched)
    nsh = nslots * CFG["wn"]
    GROUP = CFG["group"]
    AGRP = CFG["agrp"]
    WN = CFG["wn"]
    WGRP = CFG["wgrp"]
    assert WN * WGRP == 512
    stream_s = CFG["s_mode"] == "stream"
    assert nt % GROUP == 0
    gsizes = [8, 8, 8, 8] + [GROUP] * ((nt - 32) // GROUP)
    assert sum(gsizes) == nt
    g_of = []
    gstart = []
    off = 0
    for gi, gsz in enumerate(gsizes):
        gstart.append(off)
        for r in range(gsz):
            g_of.append((gi, r))
        off += gsz

    nc = bacc.Bacc()
    eax_d = nc.dram_tensor("eax", [FEAT, e_pad], F8, kind="ExternalInput")
    if stream_s:
        s_d = nc.dram_tensor("s", [P, nt * CFG["wn"]], F8, kind="ExternalInput")
    else:
        cw_d = nc.dram_tensor("cw", [P, nt], F32, kind="ExternalInput")
    blob_d = nc.dram_tensor("blob", [P, BLOB_W], BF, kind="ExternalInput")
    ub2_d = nc.dram_tensor("ub2", [64, 1], F32, kind="ExternalInput")
    xua_d = nc.dram_tensor("xua", [NODE_D, nsh], BF, kind="ExternalInput")
    xub_d = nc.dram_tensor("xub", [UPB, nsh], BF, kind="ExternalInput")
    out_d = nc.dram_tensor("out", [64, nsh], BF, kind="ExternalOutput")

    with tile.TileContext(nc) as tc:
        with (
            tc.tile_pool(name="consts", bufs=1) as consts,
            tc.tile_pool(name="geax", bufs=CFG["eax_bufs"]) as geax,
            tc.tile_pool(name="gs", bufs=CFG["s_bufs"]) as gs,
            tc.tile_pool(name="gx", bufs=CFG["gx_bufs"]) as gx,
            tc.tile_pool(name="ework", bufs=CFG["ework"]) as ework,
            tc.tile_pool(name="swork", bufs=CFG["swork"]) as swork,
            tc.tile_pool(name="nwork", bufs=CFG["nwork"]) as nwork,
            tc.tile_pool(name="gout", bufs=CFG["gout_bufs"]) as gout,
            tc.tile_pool(name="ph", bufs=CFG["ph"], space="PSUM") as ph,
            tc.tile_pool(name="pt", bufs=CFG["pt"], space="PSUM") as pt,
            tc.tile_pool(name="pr", bufs=1, space="PSUM") as pr,
            tc.tile_pool(name="p2", bufs=1, space="PSUM") as p2,
            tc.tile_pool(name="po", bufs=1, space="PSUM") as po,
        ):
            blob_t = consts.tile([P, BLOB_W], BF)
            nc.sync.dma_start(blob_t[:], blob_d[:])
            mw1_t = blob_t[0:FEAT, _B_MW1:_B_MW1 + HID]
            mw2_t = blob_t[0:HID, _B_MW2:_B_MW2 + 64]
            uw1ax_t = blob_t[0:NODE_D, _B_UW1A:_B_UW1A + HID]
            uw1ar_t = blob_t[0:NODE_D, _B_UW1R:_B_UW1R + HID]
            uw1b_t = blob_t[0:UPB, _B_UW1B:_B_UW1B + HID]
            uw2_t = blob_t[0:HID, _B_UW2:_B_UW2 + 64]
            iota_t = blob_t[:, _B_IOTA:_B_IOTA + P]
            ub2_t = consts.tile([64, 1], F32)
            nc.sync.dma_start(ub2_t[:], ub2_d[:])
            if not stream_s:
                cw_t = consts.tile([P, nt], F32)
                nc.scalar.dma_start(cw_t[:], cw_d[:])

            eaxq = CFG["eax_q"]
            sq = CFG["s_q"]
            rcycle = CFG["relu_cycle"]
            eax_g = s_g = None
            xua_g = xub_g = o_g = None
            t4_ps = None
            t = 0
            rr = 0
            for j in range(nslots):
                tj = t_sched[j]
                jg, jr = divmod(j, WGRP)
                if jr == 0:
                    nw = min(WGRP, nslots - jg * WGRP)
                    W = nw * WN
                    xua_g = gx.tile([NODE_D, WGRP * WN], BF, tag="xua")
                    getattr(nc, CFG["xua_q"]).dma_start(
                        xua_g[:, 0:W],
                        xua_d[:, jg * WGRP * WN:jg * WGRP * WN + W],
                    )

                    xub_g = gx.tile([UPB, WGRP * WN], BF, tag="xub")
                    getattr(nc, CFG["xub_q"]).dma_start(
                        xub_g[:, 0:W],
                        xub_d[:, jg * WGRP * WN:jg * WGRP * WN + W],
                    )
                    o_g = gout.tile([64, WGRP * WN], BF, tag="o")
                    t4_ps = pt.tile([HID, WGRP * WN], F32, space="PSUM")
                kdone = 0
                while kdone < tj:
                    nk = min(AGRP, tj - kdone)
                    h4_ps = ph.tile([P, AGRP * HID], F32, space="PSUM")
                    chunk = []
                    for i in range(nk):
                        g, r = g_of[t]
                        if r == 0:
                            n = gsizes[g]
                            g0 = gstart[g]
                            eax_g = geax.tile([FEAT, GROUP * P], F8, tag="eax")
                            eng = getattr(nc, eaxq[g % len(eaxq)])
                            eng.dma_start(
                                eax_g[:, 0:n * P],
                                eax_d[:, g0 * P:(g0 + n) * P],
                            )
                            if stream_s:
                                s_g = gs.tile([P, GROUP * WN], F8, tag="sg")
                                eng = getattr(nc, sq[g % len(sq)])
                                eng.dma_start(
                                    s_g[:, 0:n * WN],
                                    s_d[:, g0 * WN:(g0 + n) * WN],
                                )
                        nc.tensor.matmul(
                            h4_ps[:, i * HID:(i + 1) * HID],
                            lhsT=eax_g[:, r * P:(r + 1) * P], rhs=mw1_t,
                            start=True, stop=True,
                        )
                        if stream_s:
                            s_t = s_g[:, r * WN:(r + 1) * WN]
                        else:
                            s_t = swork.tile([P, WN], BF, tag="S")
                            nc.vector.tensor_scalar(
                                out=s_t[:], in0=iota_t[:, 0:WN],
                                scalar1=cw_t[:, t:t + 1], scalar2=0.0,
                                op0=mybir.AluOpType.is_equal,
                                op1=mybir.AluOpType.bypass,
                            )
                            s_t = s_t[:]
                        chunk.append((i, s_t))
                        t += 1
                    g4_t = ework.tile([P, AGRP * HID], F8, tag="G")
                    _relu(nc, rcycle[rr % len(rcycle)],
                          g4_t[:, 0:nk * HID], h4_ps[:, 0:nk * HID])
                    rr += 1
                    for i, s_t in chunk:
                        nc.tensor.matmul(
                            t4_ps[:, jr * WN:(jr + 1) * WN],
                            lhsT=g4_t[:, i * HID:(i + 1) * HID],
                            rhs=s_t,
                            start=(kdone + i == 0), stop=(kdone + i == tj - 1),
                        )
                    kdone += nk
                if jr == nw - 1:
                    W = nw * WN
                    hh = nwork.tile([HID, WGRP * WN], BF, tag="Hh")
                    _copy(nc, CFG["hh_eng"], hh[:, 0:W], t4_ps[:, 0:W])
                    r4_ps = pr.tile([64, WGRP * WN], F32, space="PSUM")
                    nc.tensor.matmul(r4_ps[:, 0:W], lhsT=mw2_t, rhs=hh[:, 0:W],
                                     start=True, stop=True)
                    recv_sb = nwork.tile([64, WGRP * WN], BF, tag="recv")
                    _copy(nc, CFG["recv_eng"], recv_sb[:, 0:W], r4_ps[:, 0:W])
                    h2_ps = p2.tile([HID, WGRP * WN], F32, space="PSUM")
                    nc.tensor.matmul(h2_ps[:, 0:W], lhsT=uw1ax_t,
                                     rhs=xua_g[:, 0:W], start=True, stop=False)
                    nc.tensor.matmul(h2_ps[:, 0:W], lhsT=uw1ar_t,
                                     rhs=recv_sb[:, 0:W], start=False,
                                     stop=False)
                    nc.tensor.matmul(h2_ps[:, 0:W], lhsT=uw1b_t,
                                     rhs=xub_g[:, 0:W], start=False, stop=True)
                    h2r = nwork.tile([HID, WGRP * WN], BF, tag="h2r")
                    _relu(nc, CFG["h2r_eng"], h2r[:, 0:W], h2_ps[:, 0:W])
                    o_ps = po.tile([64, WGRP * WN], F32, space="PSUM")
                    nc.tensor.matmul(o_ps[:, 0:W], lhsT=uw2_t, rhs=h2r[:, 0:W],
                                     start=True, stop=True)
                    if CFG["obias_eng"] == "scalar":
                        nc.scalar.activation(
                            o_g[:, 0:W], o_ps[:, 0:W],
                            mybir.ActivationFunctionType.Identity,
                            bias=ub2_t[:, 0:1],
                        )
                    else:
                        nc.vector.tensor_scalar(
                            out=o_g[:, 0:W], in0=o_ps[:, 0:W],
                            scalar1=ub2_t[:, 0:1], scalar2=None,
                            op0=mybir.AluOpType.add,
                        )
                    getattr(nc, CFG["out_q"]).dma_start(
                        out_d[:, jg * WGRP * WN:jg * WGRP * WN + W],
                        o_g[:, 0:W],
                    )
    nc.finalize()
    return nc


def _schedule(col, n_nodes):
    """Assign WN-node blocks to (core, slot); shared per-slot tile counts."""
    GROUP = CFG["group"]
    WN = CFG["wn"]
    WSH = WN.bit_length() - 1
    nblk = -(-n_nodes // WN)
    nslots = -(-nblk // NCORES)
    nblk_pad = nslots * NCORES
    nsh = nslots * WN

    blk = (col >> WSH).astype(np.int64)
    order = np.argsort(blk, kind="stable")
    bc = np.bincount(blk, minlength=nblk_pad)
    bstart = np.zeros(nblk_pad + 1, np.int64)
    np.cumsum(bc, out=bstart[1:])

    sorted_blocks = np.argsort(-bc, kind="stable")
    blk_assign = sorted_blocks.reshape(nslots, NCORES)   # [slot, core]
    grp_max = bc[blk_assign].max(axis=1)
    t_sched = np.maximum(1, -(-grp_max // P)).astype(np.int64)
    pad = (-int(t_sched.sum())) % GROUP
    t_sched[-1] += pad
    t_sched = [int(v) for v in t_sched]
    return t_sched, blk_assign, order, bc, bstart, nslots, nsh


def kernel(x, edge_index, edge_attr, u, node_batch, wts,
           mw1, mb1, mw2, mb2, uw1, ub1, uw2, ub2):
    x = np.asarray(x, np.float32)
    edge_index = np.asarray(edge_index)
    edge_attr = np.asarray(edge_attr, np.float32)
    u = np.asarray(u, np.float32)
    node_batch = np.asarray(node_batch).astype(np.int64)
    wts = np.asarray(wts, np.float32).reshape(-1)
    mw1 = np.asarray(mw1, np.float32)
    mb1 = np.asarray(mb1, np.float32)
    mw2 = np.asarray(mw2, np.float32)
    mb2 = np.asarray(mb2, np.float32)
    uw1 = np.asarray(uw1, np.float32)
    ub1 = np.asarray(ub1, np.float32)
    uw2 = np.asarray(uw2, np.float32)
    ub2 = np.asarray(ub2, np.float32)

    n_nodes = x.shape[0]
    row = np.asarray(edge_index[0], np.int64)
    col = np.asarray(edge_index[1], np.int64)

    sched = _schedule(col, n_nodes)
    (t_sched, blk_assign, order, bc, bstart, nslots, nsh) = sched
    nt = sum(t_sched)
    e_pad = nt * P
    stream_s = CFG["s_mode"] == "stream"

    cnt = np.bincount(col, minlength=n_nodes).astype(np.float32)
    rc = 1.0 / np.maximum(cnt, 1.0)
    wsum = np.bincount(col, weights=wts, minlength=n_nodes).astype(np.float32)
    s_node = wsum * rc

    WN = CFG["wn"]
    colof = (col & (WN - 1)).astype(np.int64)
    wrc = wts * rc[col]

    key = (tuple(t_sched), CFG["s_mode"])
    if key not in _program_cache:
        _program_cache[key] = _build_program(t_sched)
    nc = _program_cache[key]

    # const blob (shared by all cores)
    v_row = mb2 @ uw1[NODE_D:2 * NODE_D, :]              # [HID]
    blob = np.zeros((P, BLOB_W), np.float32)
    blob[0:NODE_D + EDGE_D, _B_MW1:_B_MW1 + HID] = mw1
    blob[NODE_D + EDGE_D, _B_MW1:_B_MW1 + HID] = mb1
    blob[0:HID, _B_MW2:_B_MW2 + 64] = mw2
    blob[0:2 * NODE_D, _B_UW1A:_B_UW1A + HID] = uw1[0:2 * NODE_D, :]
    blob[0:NODE_D, _B_UW1R:_B_UW1R + HID] = uw1[NODE_D:2 * NODE_D, :]
    blob[0:GLOB_D, _B_UW1B:_B_UW1B + HID] = uw1[2 * NODE_D:, :]
    blob[GLOB_D, _B_UW1B:_B_UW1B + HID] = v_row
    blob[GLOB_D + 1, _B_UW1B:_B_UW1B + HID] = ub1
    blob[0:HID, _B_UW2:_B_UW2 + 64] = uw2
    blob[:, _B_IOTA:_B_IOTA + P] = np.arange(P, dtype=np.float32)[None, :]
    blob_bf = blob.astype(bf16)
    ub2_a = ub2.reshape(64, 1).astype(np.float32)

    u_per_node = u[node_batch]                           # [N, GLOB_D]

    slot_off = np.zeros(nslots + 1, np.int64)
    np.cumsum(np.asarray(t_sched) * P, out=slot_off[1:])

    tidx_all = np.arange(e_pad) // P
    pidx_all = np.arange(e_pad) % P

    in_maps = []
    node_idx_cores = []
    for c in range(NCORES):
        eidx = np.full(e_pad, -1, np.int64)
        nidx = np.full(nsh, -1, np.int64)
        for j in range(nslots):
            b = int(blk_assign[j, c])
            m = int(bc[b])
            o = slot_off[j]
            eidx[o:o + m] = order[bstart[b]:bstart[b] + m]
            n0 = b * WN
            nn = min(WN, n_nodes - n0)
            if nn > 0:
                nidx[j * WN:j * WN + nn] = np.arange(n0, n0 + nn)
        evalid = eidx >= 0
        eidxc = np.where(evalid, eidx, 0)
        # eax: [x[row] | edge_attr | 1] * wrc, transposed, fp8
        eax = np.empty((e_pad, FEAT), np.float32)
        eax[:, 0:NODE_D] = x[row[eidxc]]
        eax[:, NODE_D:NODE_D + EDGE_D] = edge_attr[eidxc]
        eax[:, FEAT - 1] = 1.0
        wrcv = np.zeros(e_pad, np.float32)
        wrcv[evalid] = wrc[eidx[evalid]]
        eax *= wrcv[:, None]
        np.clip(eax, -240.0, 240.0, out=eax)

        im = {
            "eax": np.ascontiguousarray(eax.T).astype(f8),
            "blob": blob_bf,
            "ub2": ub2_a,
        }
        if stream_s:
            s_all = np.zeros((nt, P, WN), f8)
            cv = colof[eidx[evalid]]
            s_all[tidx_all[evalid], pidx_all[evalid], cv] = f8(1.0)
            im["s"] = np.ascontiguousarray(
                s_all.transpose(1, 0, 2).reshape(P, nt * WN))
        else:
            cwv = np.zeros(e_pad, np.float32)
            cwv[evalid] = colof[eidx[evalid]].astype(np.float32)
            cwv[~evalid] = -1.0
            im["cw"] = np.ascontiguousarray(
                cwv.reshape(nt, P).T).astype(np.float32)

        nvalid = nidx >= 0
        nidxc = np.where(nvalid, nidx, 0)
        xua = x[nidxc].astype(np.float32)
        xua[~nvalid] = 0.0
        xub = np.zeros((nsh, UPB), np.float32)
        xub[:, 0:GLOB_D] = u_per_node[nidxc]
        xub[:, GLOB_D] = s_node[nidxc]
        xub[:, GLOB_D + 1] = 1.0
        xub[~nvalid] = 0.0
        im["xua"] = np.ascontiguousarray(xua.T).astype(bf16)
        im["xub"] = np.ascontiguousarray(xub.T).astype(bf16)

        in_maps.append(im)
        node_idx_cores.append((nidx, nvalid))

    res = run_bass_kernel_spmd(nc, in_maps, core_ids=list(range(NCORES)))
    global _last_results
    _last_results = res

    out_full = np.zeros((n_nodes, 64), np.float32)
    for c in range(NCORES):
        nidx, nvalid = node_idx_cores[c]
        oc = np.asarray(res.results[c]["out"], dtype=np.float32)  # [64, nsh]
        out_full[nidx[nvalid]] = oc.T[nvalid]
    return out_full
